# revision 5
# baseline (speedup 1.0000x reference)
"""Block-quantized FP8 linear (KLinearFP8) on 8 trn2 NeuronCores.

y[m, n] = sum_k x_dq[m, k] * w_dq[n, k]
  x_dq: per-(row, 128-block) fp8e4m3fn-simulated quantization of x
  w_dq: weight (fp8 values held in fp32) * per-128x128-block scale

Sharding: column-parallel. weight/weight_scale_inv split along N across 8
cores, x replicated; each core computes y[:, c*2048:(c+1)*2048].

Host-side prep (layout only, values exact): the weight shard ships
pre-transposed [K, NSH] as TRN-safe fp8 (w/2 in ml_dtypes.float8_e4m3 —
all values <=224 so the cast is lossless). On device each k-slab is one
contiguous DMA straight into the K-on-partitions layout the PE needs, so
the tensor engine runs ONLY the GEMM: no PE transposes, no weight-prep
phase blocking the pipeline.

Per-core kernel: dequantize w to bf16 with one multiply (2*ws folded in),
quantize+dequantize x per (row, 128-block) with scale amax/224 (power-of-
two rescale of the reference amax/448 grid -> identical rounding), XBAR-
transpose x to K-on-partitions, then a bf16 GEMM with fp32 PSUM
accumulation, kb-outer so each stationary x-tile is reused across the 4
n-chunks and all 8 PSUM banks alternate between consecutive m-tiles.
y is written bf16 and widened on host.
"""

import numpy as np

M, K, N = 4096, 4096, 16384
NCORES = 8
NSH = N // NCORES          # 2048 columns of y per core
P = 128
KB = K // P                # 32 k-blocks
MT = M // P                # 32 m-tiles
NB = NSH // P              # 16 n-blocks per core
CHW = 512
NCH = NSH // CHW           # 4 psum chunks of 512
FP8_SAFE = 224.0           # 448/2: fits TRN e4m3 (max 240), same rounding grid

_NC_CACHE = {}


def _build(M=M, K=K, NSH=NSH, debug=False):
    import concourse.bass as bass  # noqa: F401
    import concourse.mybir as mybir
    import concourse.tile as tile
    from concourse import bacc

    KB = K // P
    MT = M // P
    NB = NSH // P
    CHW = min(512, NSH)
    NCH = NSH // CHW
    # x quantization processed in NSPL k-groups per m-tile (shorter dep
    # chains -> deeper prefetch overlap with the matmul stream).
    NSPL = 4 if (KB % 4 == 0 and KB >= 8) else 1
    KH = KB // NSPL

    f32, bf16, f8 = mybir.dt.float32, mybir.dt.bfloat16, mybir.dt.float8e4

    nc = bacc.Bacc(None, target_bir_lowering=False, debug=debug)
    x_d = nc.declare_dram_parameter("x", [M, K], f32, isOutput=False)
    wt8_d = nc.declare_dram_parameter("wt8", [K, NSH], f8, isOutput=False)
    ws_d = nc.declare_dram_parameter("ws", [NB, KB], f32, isOutput=False)
    y_d = nc.declare_dram_parameter("y", [M, NSH], bf16, isOutput=True)

    with tile.TileContext(nc) as tc:
        with (
            tc.tile_pool(name="const", bufs=1) as const,
            tc.tile_pool(name="wt", bufs=1) as wtp,
            tc.tile_pool(name="w8p", bufs=4) as w8p,
            tc.tile_pool(name="xpool", bufs=3) as xpool,
            tc.tile_pool(name="xq", bufs=3) as xqp,
            tc.tile_pool(name="xdq", bufs=3) as xdqp,
            tc.tile_pool(name="xtp", bufs=8) as xtp,
            tc.tile_pool(name="scales", bufs=8) as spool,
            tc.tile_pool(name="ypool", bufs=6) as ypool,
            tc.tile_pool(name="psum", bufs=8, space="PSUM") as psum,
        ):
            # ---- weight-block scales * 2 (undoes the host /2),
            # broadcast to all partitions: wsb[p, nb, kb] = 2*ws[nb, kb].
            ws_row = const.tile([1, NB * KB], f32)
            nc.sync.dma_start(
                ws_row[:], ws_d[:].rearrange("a b -> (a b)")[None, :]
            )
            nc.vector.tensor_scalar_mul(ws_row[:], ws_row[:], 2.0)
            wsb = const.tile([P, NB, KB], f32)
            nc.gpsimd.partition_broadcast(
                wsb[:].rearrange("p a b -> p (a b)"), ws_row[:]
            )

            # ---- weight prep: one contiguous DMA per k-slab (already
            # K-on-partitions), one dequant multiply to bf16. No PE work.
            wTs = []
            for kb in range(KB):
                w8 = w8p.tile([P, NB, P], f8, tag="w8")
                nc.scalar.dma_start(
                    w8[:].rearrange("p a b -> p (a b)"),
                    wt8_d[kb * P:(kb + 1) * P, :],
                )
                wT = wtp.tile([P, NB, P], bf16, tag=f"wT{kb}")
                nc.gpsimd.tensor_tensor(
                    wT[:], w8[:],
                    wsb[:, :, kb, None].to_broadcast((P, NB, P)),
                    mybir.AluOpType.mult,
                )
                wTs.append(wT)

            # ---- per m-tile: quantize+dequantize x (NSPL k-groups),
            # XBAR-transpose, then KB*NCH bf16 matmuls, kb-outer.
            for mt in range(MT):
                ms = slice(mt * P, (mt + 1) * P)
                xTq = []
                for q in range(NSPL):
                    ks = slice(q * KH * P, (q + 1) * KH * P)
                    xrow = xpool.tile([P, KH, P], f32, tag="xrow")
                    nc.scalar.dma_start(
                        xrow[:],
                        x_d[ms, ks].rearrange("m (kb x) -> m kb x", x=P),
                    )
                    sc = spool.tile([P, 2, KH], f32, tag="sc")
                    s2, rinv = sc[:, 0, :], sc[:, 1, :]
                    nc.vector.tensor_reduce(
                        s2, xrow[:], axis=mybir.AxisListType.X,
                        op=mybir.AluOpType.max, apply_absolute_value=True,
                    )
                    nc.vector.tensor_scalar_mul(s2, s2, float(1.0 / FP8_SAFE))
                    nc.vector.reciprocal(rinv, s2)
                    xq = xqp.tile([P, KH, P], f8, tag="xq")
                    nc.vector.tensor_tensor(
                        xq[:], xrow[:],
                        rinv[:, :, None].to_broadcast((P, KH, P)),
                        mybir.AluOpType.mult,
                    )
                    xdq = xdqp.tile([P, KH, P], bf16, tag="xdq")
                    nc.gpsimd.tensor_tensor(
                        xdq[:], xq[:],
                        s2[:, :, None].to_broadcast((P, KH, P)),
                        mybir.AluOpType.mult,
                    )
                    xT = xtp.tile([P, KH, P], bf16, tag="xT")
                    nc.sync.dma_start_transpose(
                        xT[:], xdq[:].rearrange("p a b -> p (a b)")
                    )
                    xTq.append(xT)

                pts = [
                    psum.tile([P, CHW], mybir.dt.float32, name=f"pt{c}", tag="pt")
                    for c in range(NCH)
                ]
                for kb in range(KB):
                    q, kbl = divmod(kb, KH)
                    wv = wTs[kb][:].rearrange("p a b -> p (a b)")
                    for c in range(NCH):
                        nc.tensor.matmul(
                            pts[c][:],
                            xTq[q][:, kbl, :],
                            wv[:, c * CHW:(c + 1) * CHW],
                            start=(kb == 0),
                            stop=(kb == KB - 1),
                        )
                for c in range(NCH):
                    yt = ypool.tile([P, CHW], bf16, tag="yt")
                    nc.scalar.activation(
                        yt[:], pts[c][:], mybir.ActivationFunctionType.Copy
                    )
                    nc.scalar.dma_start(y_d[ms, c * CHW:(c + 1) * CHW], yt[:])

    nc.compile()
    return nc


def _core_inputs(x, weight, ws, c, nsh=NSH, nb=NB):
    """Shard + lay out inputs for core c. Layout/dtype transforms only:
    the fp8 cast of w/2 is exact (all values <= 224)."""
    import ml_dtypes

    wsl = weight[c * nsh:(c + 1) * nsh]
    wt8 = np.ascontiguousarray(
        (wsl.T * np.float32(0.5)).astype(ml_dtypes.float8_e4m3)
    )
    return {
        "x": x,
        "wt8": wt8,
        "ws": np.ascontiguousarray(ws[c * nb:(c + 1) * nb]),
    }


def kernel(x, weight, weight_scale_inv):
    from concourse.bass_utils import run_bass_kernel_spmd

    if "nc" not in _NC_CACHE:
        _NC_CACHE["nc"] = _build()
    nc = _NC_CACHE["nc"]

    x = np.ascontiguousarray(np.asarray(x, dtype=np.float32))
    weight = np.asarray(weight, dtype=np.float32)
    ws = np.asarray(weight_scale_inv, dtype=np.float32)

    in_maps = [_core_inputs(x, weight, ws, c) for c in range(NCORES)]
    res = run_bass_kernel_spmd(nc, in_maps, list(range(NCORES)))
    y = np.concatenate(
        [np.asarray(res.results[c]["y"]) for c in range(NCORES)], axis=1
    )
    return y.astype(np.float32)


# revision 6
# speedup vs baseline: 1.0224x; 1.0224x over previous
"""Block-quantized FP8 linear (KLinearFP8) on 8 trn2 NeuronCores.

y[m, n] = sum_k x_dq[m, k] * w_dq[n, k]
  x_dq: per-(row, 128-block) fp8e4m3fn-simulated quantization of x
  w_dq: weight (fp8 values held in fp32) * per-128x128-block scale

Sharding: column-parallel. weight/weight_scale_inv split along N across 8
cores, x replicated; each core computes y[:, c*2048:(c+1)*2048].

Host-side prep (layout only, values exact): the weight shard ships
pre-transposed [K, NSH] as TRN-safe fp8 (w/2 in ml_dtypes.float8_e4m3 —
all values <=224 so the cast is lossless). On device each k-slab is one
contiguous DMA straight into the K-on-partitions layout the PE needs, so
the tensor engine runs ONLY the GEMM: no PE transposes, no weight-prep
phase blocking the pipeline.

Per-core kernel: dequantize w to bf16 with one multiply (2*ws folded in),
quantize+dequantize x per (row, 128-block) with scale amax/224 (power-of-
two rescale of the reference amax/448 grid -> identical rounding), XBAR-
transpose x to K-on-partitions, then a bf16 GEMM with fp32 PSUM
accumulation, kb-outer so each stationary x-tile is reused across the 4
n-chunks and all 8 PSUM banks alternate between consecutive m-tiles.
y is written bf16 and widened on host.
"""

import numpy as np

M, K, N = 4096, 4096, 16384
NCORES = 8
NSH = N // NCORES          # 2048 columns of y per core
P = 128
KB = K // P                # 32 k-blocks
MT = M // P                # 32 m-tiles
NB = NSH // P              # 16 n-blocks per core
CHW = 512
NCH = NSH // CHW           # 4 psum chunks of 512
FP8_SAFE = 224.0           # 448/2: fits TRN e4m3 (max 240), same rounding grid

_NC_CACHE = {}


def _build(M=M, K=K, NSH=NSH, debug=False):
    import concourse.bass as bass  # noqa: F401
    import concourse.mybir as mybir
    import concourse.tile as tile
    from concourse import bacc

    KB = K // P
    MT = M // P
    NB = NSH // P
    CHW = min(512, NSH)
    NCH = NSH // CHW
    # x quantization processed in NSPL k-groups per m-tile (shorter dep
    # chains -> deeper prefetch overlap with the matmul stream).
    NSPL = 4 if (KB % 4 == 0 and KB >= 8) else 1
    KH = KB // NSPL

    f32, bf16, f8 = mybir.dt.float32, mybir.dt.bfloat16, mybir.dt.float8e4

    nc = bacc.Bacc(None, target_bir_lowering=False, debug=debug)
    x_d = nc.declare_dram_parameter("x", [M, K], f32, isOutput=False)
    wt8_d = nc.declare_dram_parameter("wt8", [K, NSH], f8, isOutput=False)
    ws_d = nc.declare_dram_parameter("ws", [NB, KB], f32, isOutput=False)
    y_d = nc.declare_dram_parameter("y", [M, NSH], bf16, isOutput=True)

    with tile.TileContext(nc) as tc:
        with (
            tc.tile_pool(name="const", bufs=1) as const,
            tc.tile_pool(name="wt", bufs=1) as wtp,
            tc.tile_pool(name="w8p", bufs=4) as w8p,
            tc.tile_pool(name="xpool", bufs=3) as xpool,
            tc.tile_pool(name="xq", bufs=3) as xqp,
            tc.tile_pool(name="xdq", bufs=3) as xdqp,
            tc.tile_pool(name="xtp", bufs=8) as xtp,
            tc.tile_pool(name="scales", bufs=8) as spool,
            tc.tile_pool(name="ypool", bufs=6) as ypool,
            tc.tile_pool(name="psum", bufs=8, space="PSUM") as psum,
        ):
            # ---- weight-block scales * 2 (undoes the host /2),
            # broadcast to all partitions: wsb[p, nb, kb] = 2*ws[nb, kb].
            ws_row = const.tile([1, NB * KB], f32)
            nc.sync.dma_start(
                ws_row[:], ws_d[:].rearrange("a b -> (a b)")[None, :]
            )
            nc.vector.tensor_scalar_mul(ws_row[:], ws_row[:], 2.0)
            wsb = const.tile([P, NB, KB], f32)
            nc.gpsimd.partition_broadcast(
                wsb[:].rearrange("p a b -> p (a b)"), ws_row[:]
            )

            # ---- weight prep: one contiguous DMA per k-slab (already
            # K-on-partitions), one dequant multiply to bf16. No PE work.
            # Dequants split DVE/GpSimd so neither engine's FIFO starves
            # the x-prep chain at startup.
            wTs = [None] * KB

            def w_prep(kb):
                w8 = w8p.tile([P, NB, P], f8, name="w8", tag="w8")
                nc.scalar.dma_start(
                    w8[:].rearrange("p a b -> p (a b)"),
                    wt8_d[kb * P:(kb + 1) * P, :],
                )
                wT = wtp.tile([P, NB, P], bf16, name="wT", tag=f"wT{kb}")
                eng = nc.vector if kb % 8 < 5 else nc.gpsimd
                eng.tensor_tensor(
                    wT[:], w8[:],
                    wsb[:, :, kb, None].to_broadcast((P, NB, P)),
                    mybir.AluOpType.mult,
                )
                wTs[kb] = wT

            # ---- x-prep for one m-tile: quantize+dequantize (NSPL
            # k-groups), XBAR-transpose to K-on-partitions.
            def x_prep(mt):
                ms = slice(mt * P, (mt + 1) * P)
                xTq = []
                for q in range(NSPL):
                    ks = slice(q * KH * P, (q + 1) * KH * P)
                    xrow = xpool.tile([P, KH, P], f32, name="xrow", tag="xrow")
                    nc.scalar.dma_start(
                        xrow[:],
                        x_d[ms, ks].rearrange("m (kb x) -> m kb x", x=P),
                    )
                    sc = spool.tile([P, 2, KH], f32, name="sc", tag="sc")
                    s2, rinv = sc[:, 0, :], sc[:, 1, :]
                    nc.vector.tensor_reduce(
                        s2, xrow[:], axis=mybir.AxisListType.X,
                        op=mybir.AluOpType.max, apply_absolute_value=True,
                    )
                    nc.vector.tensor_scalar_mul(s2, s2, float(1.0 / FP8_SAFE))
                    nc.vector.reciprocal(rinv, s2)
                    xq = xqp.tile([P, KH, P], f8, name="xq", tag="xq")
                    nc.vector.tensor_tensor(
                        xq[:], xrow[:],
                        rinv[:, :, None].to_broadcast((P, KH, P)),
                        mybir.AluOpType.mult,
                    )
                    xdq = xdqp.tile([P, KH, P], bf16, name="xdq", tag="xdq")
                    nc.gpsimd.tensor_tensor(
                        xdq[:], xq[:],
                        s2[:, :, None].to_broadcast((P, KH, P)),
                        mybir.AluOpType.mult,
                    )
                    xT = xtp.tile([P, KH, P], bf16, name="xT", tag="xT")
                    nc.sync.dma_start_transpose(
                        xT[:], xdq[:].rearrange("p a b -> p (a b)")
                    )
                    xTq.append(xT)
                return xTq

            def drains(mt, pts):
                ms = slice(mt * P, (mt + 1) * P)
                for c in range(NCH):
                    yt = ypool.tile([P, CHW], bf16, name="yt", tag="yt")
                    nc.scalar.activation(
                        yt[:], pts[c][:], mybir.ActivationFunctionType.Copy
                    )
                    nc.scalar.dma_start(y_d[ms, c * CHW:(c + 1) * CHW], yt[:])

            # ---- software-pipelined main loop: x-prep runs one m-tile
            # ahead of its matmuls; PSUM drains trail one m-tile so their
            # matmul-completion waits never block the scalar engine's
            # x-load queue.
            xT_next = x_prep(0)
            for kb in range(KB):
                w_prep(kb)
            prev = None       # (mt, pts) awaiting drain
            for mt in range(MT):
                xTq = xT_next
                if mt + 1 < MT:
                    xT_next = x_prep(mt + 1)
                if prev is not None:
                    drains(*prev)
                pts = [
                    psum.tile([P, CHW], mybir.dt.float32, name=f"pt{c}", tag="pt")
                    for c in range(NCH)
                ]
                for kb in range(KB):
                    q, kbl = divmod(kb, KH)
                    wv = wTs[kb][:].rearrange("p a b -> p (a b)")
                    for c in range(NCH):
                        nc.tensor.matmul(
                            pts[c][:],
                            xTq[q][:, kbl, :],
                            wv[:, c * CHW:(c + 1) * CHW],
                            start=(kb == 0),
                            stop=(kb == KB - 1),
                        )
                prev = (mt, pts)
            drains(*prev)

    nc.compile()
    return nc


def _core_inputs(x, weight, ws, c, nsh=NSH, nb=NB):
    """Shard + lay out inputs for core c. Layout/dtype transforms only:
    the fp8 cast of w/2 is exact (all values <= 224)."""
    import ml_dtypes

    wsl = weight[c * nsh:(c + 1) * nsh]
    wt8 = np.ascontiguousarray(
        (wsl.T * np.float32(0.5)).astype(ml_dtypes.float8_e4m3)
    )
    return {
        "x": x,
        "wt8": wt8,
        "ws": np.ascontiguousarray(ws[c * nb:(c + 1) * nb]),
    }


def kernel(x, weight, weight_scale_inv):
    from concourse.bass_utils import run_bass_kernel_spmd

    if "nc" not in _NC_CACHE:
        _NC_CACHE["nc"] = _build()
    nc = _NC_CACHE["nc"]

    x = np.ascontiguousarray(np.asarray(x, dtype=np.float32))
    weight = np.asarray(weight, dtype=np.float32)
    ws = np.asarray(weight_scale_inv, dtype=np.float32)

    in_maps = [_core_inputs(x, weight, ws, c) for c in range(NCORES)]
    res = run_bass_kernel_spmd(nc, in_maps, list(range(NCORES)))
    y = np.concatenate(
        [np.asarray(res.results[c]["y"]) for c in range(NCORES)], axis=1
    )
    return y.astype(np.float32)


# revision 7
# speedup vs baseline: 1.0781x; 1.0546x over previous
"""Block-quantized FP8 linear (KLinearFP8) on 8 trn2 NeuronCores.

y[m, n] = sum_k x_dq[m, k] * w_dq[n, k]
  x_dq: per-(row, 128-block) fp8e4m3fn-simulated quantization of x
  w_dq: weight (fp8 values held in fp32) * per-128x128-block scale

Sharding: column-parallel. weight/weight_scale_inv split along N across 8
cores, x replicated; each core computes y[:, c*2048:(c+1)*2048].

Host-side prep (layout only, values exact): the weight shard ships
pre-transposed [K, NSH] as TRN-safe fp8 (w/2 in ml_dtypes.float8_e4m3 —
all values <=224 so the cast is lossless). On device each k-slab is one
contiguous DMA straight into the K-on-partitions layout the PE needs, so
the tensor engine runs ONLY the GEMM: no PE transposes, no weight-prep
phase blocking the pipeline.

Per-core kernel: dequantize w to bf16 with one multiply (2*ws folded in),
quantize+dequantize x per (row, 128-block) with scale amax/224 (power-of-
two rescale of the reference amax/448 grid -> identical rounding), XBAR-
transpose x to K-on-partitions, then a bf16 GEMM with fp32 PSUM
accumulation, kb-outer so each stationary x-tile is reused across the 4
n-chunks and all 8 PSUM banks alternate between consecutive m-tiles.
y is written bf16 and widened on host.
"""

import numpy as np

M, K, N = 4096, 4096, 16384
NCORES = 8
NSH = N // NCORES          # 2048 columns of y per core
P = 128
KB = K // P                # 32 k-blocks
MT = M // P                # 32 m-tiles
NB = NSH // P              # 16 n-blocks per core
CHW = 512
NCH = NSH // CHW           # 4 psum chunks of 512
FP8_SAFE = 224.0           # 448/2: fits TRN e4m3 (max 240), same rounding grid

_NC_CACHE = {}


def _build(M=M, K=K, NSH=NSH, debug=False):
    import concourse.bass as bass  # noqa: F401
    import concourse.mybir as mybir
    import concourse.tile as tile
    from concourse import bacc

    KB = K // P
    MT = M // P
    NB = NSH // P
    CHW = min(512, NSH)
    NCH = NSH // CHW
    # x quantization processed in NSPL k-groups per m-tile (shorter dep
    # chains -> deeper prefetch overlap with the matmul stream).
    NSPL = 4 if (KB % 4 == 0 and KB >= 8) else 1
    KH = KB // NSPL

    f32, bf16, f8 = mybir.dt.float32, mybir.dt.bfloat16, mybir.dt.float8e4

    nc = bacc.Bacc(None, target_bir_lowering=False, debug=debug)
    x_d = nc.declare_dram_parameter("x", [M, K], f32, isOutput=False)
    wt8_d = nc.declare_dram_parameter("wt8", [K, NSH], f8, isOutput=False)
    ws_d = nc.declare_dram_parameter("ws", [NB, KB], f32, isOutput=False)
    y_d = nc.declare_dram_parameter("y", [M, NSH], bf16, isOutput=True)

    with tile.TileContext(nc) as tc:
        with (
            tc.tile_pool(name="const", bufs=1) as const,
            tc.tile_pool(name="wt", bufs=1) as wtp,
            tc.tile_pool(name="w8p", bufs=2) as w8p,
            tc.tile_pool(name="xpool", bufs=4) as xpool,
            tc.tile_pool(name="xq", bufs=4) as xqp,
            tc.tile_pool(name="xdq", bufs=3) as xdqp,
            tc.tile_pool(name="xtp", bufs=12) as xtp,
            tc.tile_pool(name="scales", bufs=8) as spool,
            tc.tile_pool(name="ypool", bufs=4) as ypool,
            tc.tile_pool(name="psum", bufs=8, space="PSUM") as psum,
        ):
            # ---- weight-block scales * 2 (undoes the host /2),
            # broadcast to all partitions: wsb[p, nb, kb] = 2*ws[nb, kb].
            ws_row = const.tile([1, NB * KB], f32)
            nc.sync.dma_start(
                ws_row[:], ws_d[:].rearrange("a b -> (a b)")[None, :]
            )
            nc.vector.tensor_scalar_mul(ws_row[:], ws_row[:], 2.0)
            wsb = const.tile([P, NB, KB], f32)
            nc.gpsimd.partition_broadcast(
                wsb[:].rearrange("p a b -> p (a b)"), ws_row[:]
            )

            # ---- weight prep: one contiguous DMA per k-slab (already
            # K-on-partitions), one dequant multiply to bf16. No PE work.
            # Dequants split DVE/GpSimd so neither engine's FIFO starves
            # the x-prep chain at startup.
            wTs = [None] * KB

            def w_prep(kb):
                w8 = w8p.tile([P, NB, P], f8, name="w8", tag="w8")
                nc.scalar.dma_start(
                    w8[:].rearrange("p a b -> p (a b)"),
                    wt8_d[kb * P:(kb + 1) * P, :],
                )
                wT = wtp.tile([P, NB, P], bf16, name="wT", tag=f"wT{kb}")
                eng = nc.vector if kb % 8 < 5 else nc.gpsimd
                eng.tensor_tensor(
                    wT[:], w8[:],
                    wsb[:, :, kb, None].to_broadcast((P, NB, P)),
                    mybir.AluOpType.mult,
                )
                wTs[kb] = wT

            # ---- x-prep for one m-tile: quantize+dequantize (NSPL
            # k-groups), XBAR-transpose to K-on-partitions.
            def x_prep(mt):
                ms = slice(mt * P, (mt + 1) * P)
                xTq = []
                for q in range(NSPL):
                    ks = slice(q * KH * P, (q + 1) * KH * P)
                    xrow = xpool.tile([P, KH, P], f32, name="xrow", tag="xrow")
                    nc.scalar.dma_start(
                        xrow[:],
                        x_d[ms, ks].rearrange("m (kb x) -> m kb x", x=P),
                    )
                    sc = spool.tile([P, 2, KH], f32, name="sc", tag="sc")
                    s2, rinv = sc[:, 0, :], sc[:, 1, :]
                    nc.vector.tensor_reduce(
                        s2, xrow[:], axis=mybir.AxisListType.X,
                        op=mybir.AluOpType.max, apply_absolute_value=True,
                    )
                    nc.vector.tensor_scalar_mul(s2, s2, float(1.0 / FP8_SAFE))
                    nc.vector.reciprocal(rinv, s2)
                    xq = xqp.tile([P, KH, P], f8, name="xq", tag="xq")
                    nc.vector.tensor_tensor(
                        xq[:], xrow[:],
                        rinv[:, :, None].to_broadcast((P, KH, P)),
                        mybir.AluOpType.mult,
                    )
                    xdq = xdqp.tile([P, KH, P], bf16, name="xdq", tag="xdq")
                    nc.vector.tensor_tensor(
                        xdq[:], xq[:],
                        s2[:, :, None].to_broadcast((P, KH, P)),
                        mybir.AluOpType.mult,
                    )
                    xT = xtp.tile([P, KH, P], bf16, name="xT", tag="xT")
                    nc.sync.dma_start_transpose(
                        xT[:], xdq[:].rearrange("p a b -> p (a b)")
                    )
                    xTq.append(xT)
                return xTq

            def drains(mt, pts):
                ms = slice(mt * P, (mt + 1) * P)
                for c in range(NCH):
                    yt = ypool.tile([P, CHW], bf16, name="yt", tag="yt")
                    nc.scalar.activation(
                        yt[:], pts[c][:], mybir.ActivationFunctionType.Copy
                    )
                    nc.scalar.dma_start(y_d[ms, c * CHW:(c + 1) * CHW], yt[:])

            # ---- software-pipelined main loop: x-prep runs one m-tile
            # ahead of its matmuls; PSUM drains trail one m-tile so their
            # matmul-completion waits never block the scalar engine's
            # x-load queue.
            xT_next = x_prep(0)
            for kb in range(KB):
                w_prep(kb)
            prev = None       # (mt, pts) awaiting drain
            for mt in range(MT):
                xTq = xT_next
                if mt + 1 < MT:
                    xT_next = x_prep(mt + 1)
                if prev is not None:
                    drains(*prev)
                pts = [
                    psum.tile([P, CHW], mybir.dt.float32, name=f"pt{c}", tag="pt")
                    for c in range(NCH)
                ]
                for kb in range(KB):
                    q, kbl = divmod(kb, KH)
                    wv = wTs[kb][:].rearrange("p a b -> p (a b)")
                    for c in range(NCH):
                        nc.tensor.matmul(
                            pts[c][:],
                            xTq[q][:, kbl, :],
                            wv[:, c * CHW:(c + 1) * CHW],
                            start=(kb == 0),
                            stop=(kb == KB - 1),
                        )
                prev = (mt, pts)
            drains(*prev)

    nc.compile()
    return nc


def _core_inputs(x, weight, ws, c, nsh=NSH, nb=NB):
    """Shard + lay out inputs for core c. Layout/dtype transforms only:
    the fp8 cast of w/2 is exact (all values <= 224)."""
    import ml_dtypes

    wsl = weight[c * nsh:(c + 1) * nsh]
    wt8 = np.ascontiguousarray(
        (wsl.T * np.float32(0.5)).astype(ml_dtypes.float8_e4m3)
    )
    return {
        "x": x,
        "wt8": wt8,
        "ws": np.ascontiguousarray(ws[c * nb:(c + 1) * nb]),
    }


def kernel(x, weight, weight_scale_inv):
    from concourse.bass_utils import run_bass_kernel_spmd

    if "nc" not in _NC_CACHE:
        _NC_CACHE["nc"] = _build()
    nc = _NC_CACHE["nc"]

    x = np.ascontiguousarray(np.asarray(x, dtype=np.float32))
    weight = np.asarray(weight, dtype=np.float32)
    ws = np.asarray(weight_scale_inv, dtype=np.float32)

    in_maps = [_core_inputs(x, weight, ws, c) for c in range(NCORES)]
    res = run_bass_kernel_spmd(nc, in_maps, list(range(NCORES)))
    y = np.concatenate(
        [np.asarray(res.results[c]["y"]) for c in range(NCORES)], axis=1
    )
    return y.astype(np.float32)


# revision 9
# speedup vs baseline: 1.0859x; 1.0072x over previous
"""Block-quantized FP8 linear (KLinearFP8) on 8 trn2 NeuronCores.

y[m, n] = sum_k x_dq[m, k] * w_dq[n, k]
  x_dq: per-(row, 128-block) fp8e4m3fn-simulated quantization of x
  w_dq: weight (fp8 values held in fp32) * per-128x128-block scale

Sharding: column-parallel. weight/weight_scale_inv split along N across 8
cores, x replicated; each core computes y[:, c*2048:(c+1)*2048].

Host-side prep (layout only, values exact): the weight shard ships
pre-transposed [K, NSH] as TRN-safe fp8 (w/2 in ml_dtypes.float8_e4m3 —
all values <=224 so the cast is lossless). On device each k-slab is one
contiguous DMA straight into the K-on-partitions layout the PE needs, so
the tensor engine runs ONLY the GEMM: no PE transposes, no weight-prep
phase blocking the pipeline.

Per-core kernel: dequantize w to bf16 with one multiply (2*ws folded in),
quantize+dequantize x per (row, 128-block) with scale amax/224 (power-of-
two rescale of the reference amax/448 grid -> identical rounding), XBAR-
transpose x to K-on-partitions, then a bf16 GEMM with fp32 PSUM
accumulation, kb-outer so each stationary x-tile is reused across the 4
n-chunks and all 8 PSUM banks alternate between consecutive m-tiles.
y is written bf16 and widened on host.
"""

import numpy as np

M, K, N = 4096, 4096, 16384
NCORES = 8
NSH = N // NCORES          # 2048 columns of y per core
P = 128
KB = K // P                # 32 k-blocks
MT = M // P                # 32 m-tiles
NB = NSH // P              # 16 n-blocks per core
CHW = 512
NCH = NSH // CHW           # 4 psum chunks of 512
FP8_SAFE = 224.0           # 448/2: fits TRN e4m3 (max 240), same rounding grid

_NC_CACHE = {}


def _build(M=M, K=K, NSH=NSH, debug=False):
    import concourse.bass as bass  # noqa: F401
    import concourse.mybir as mybir
    import concourse.tile as tile
    from concourse import bacc

    KB = K // P
    MT = M // P
    NB = NSH // P
    CHW = min(512, NSH)
    NCH = NSH // CHW
    # x quantization processed in NSPL k-groups per m-tile (shorter dep
    # chains -> deeper prefetch overlap with the matmul stream).
    NSPL = 4 if (KB % 4 == 0 and KB >= 8) else 1
    KH = KB // NSPL

    f32, bf16, f8 = mybir.dt.float32, mybir.dt.bfloat16, mybir.dt.float8e4

    nc = bacc.Bacc(None, target_bir_lowering=False, debug=debug)
    x_d = nc.declare_dram_parameter("x", [M, K], f32, isOutput=False)
    wt8_d = nc.declare_dram_parameter("wt8", [K, NSH], f8, isOutput=False)
    ws_d = nc.declare_dram_parameter("ws", [NB, KB], f32, isOutput=False)
    y_d = nc.declare_dram_parameter("y", [M, NSH], bf16, isOutput=True)

    with tile.TileContext(nc) as tc:
        with (
            tc.tile_pool(name="const", bufs=1) as const,
            tc.tile_pool(name="wt", bufs=1) as wtp,
            tc.tile_pool(name="w8p", bufs=2) as w8p,
            tc.tile_pool(name="xpool", bufs=4) as xpool,
            tc.tile_pool(name="xq", bufs=4) as xqp,
            tc.tile_pool(name="xdq", bufs=3) as xdqp,
            tc.tile_pool(name="xtp", bufs=12) as xtp,
            tc.tile_pool(name="scales", bufs=8) as spool,
            tc.tile_pool(name="ypool", bufs=4) as ypool,
            tc.tile_pool(name="psum", bufs=8, space="PSUM") as psum,
        ):
            # ---- weight-block scales * 2 (undoes the host /2),
            # broadcast to all partitions: wsb[p, nb, kb] = 2*ws[nb, kb].
            ws_row = const.tile([1, NB * KB], f32)
            nc.sync.dma_start(
                ws_row[:], ws_d[:].rearrange("a b -> (a b)")[None, :]
            )
            nc.vector.tensor_scalar_mul(ws_row[:], ws_row[:], 2.0)
            wsb = const.tile([P, NB, KB], f32)
            nc.gpsimd.partition_broadcast(
                wsb[:].rearrange("p a b -> p (a b)"), ws_row[:]
            )

            # ---- weight prep: one contiguous DMA per k-slab (already
            # K-on-partitions), one dequant multiply to bf16. No PE work.
            # Dequants split DVE/GpSimd so neither engine's FIFO starves
            # the x-prep chain at startup.
            wTs = [None] * KB

            def w_prep(kb):
                w8 = w8p.tile([P, NB, P], f8, name="w8", tag="w8")
                nc.scalar.dma_start(
                    w8[:].rearrange("p a b -> p (a b)"),
                    wt8_d[kb * P:(kb + 1) * P, :],
                )
                wT = wtp.tile([P, NB, P], bf16, name="wT", tag=f"wT{kb}")
                eng = nc.vector if kb % 8 < 5 else nc.gpsimd
                eng.tensor_tensor(
                    wT[:], w8[:],
                    wsb[:, :, kb, None].to_broadcast((P, NB, P)),
                    mybir.AluOpType.mult,
                )
                wTs[kb] = wT

            # ---- x-prep for one m-tile: quantize+dequantize (NSPL
            # k-groups), XBAR-transpose to K-on-partitions.
            def x_prep(mt):
                ms = slice(mt * P, (mt + 1) * P)
                xTq = []
                for q in range(NSPL):
                    ks = slice(q * KH * P, (q + 1) * KH * P)
                    xrow = xpool.tile([P, KH, P], f32, name="xrow", tag="xrow")
                    nc.scalar.dma_start(
                        xrow[:],
                        x_d[ms, ks].rearrange("m (kb x) -> m kb x", x=P),
                    )
                    sc = spool.tile([P, 2, KH], f32, name="sc", tag="sc")
                    s2, rinv = sc[:, 0, :], sc[:, 1, :]
                    nc.vector.tensor_reduce(
                        s2, xrow[:], axis=mybir.AxisListType.X,
                        op=mybir.AluOpType.max, apply_absolute_value=True,
                    )
                    nc.vector.tensor_scalar_mul(s2, s2, float(1.0 / FP8_SAFE))
                    nc.vector.reciprocal(rinv, s2)
                    xq = xqp.tile([P, KH, P], f8, name="xq", tag="xq")
                    nc.vector.tensor_tensor(
                        xq[:], xrow[:],
                        rinv[:, :, None].to_broadcast((P, KH, P)),
                        mybir.AluOpType.mult,
                    )
                    xdq = xdqp.tile([P, KH, P], bf16, name="xdq", tag="xdq")
                    nc.vector.tensor_tensor(
                        xdq[:], xq[:],
                        s2[:, :, None].to_broadcast((P, KH, P)),
                        mybir.AluOpType.mult,
                    )
                    xT = xtp.tile([P, KH, P], bf16, name="xT", tag="xT")
                    nc.sync.dma_start_transpose(
                        xT[:], xdq[:].rearrange("p a b -> p (a b)")
                    )
                    xTq.append(xT)
                return xTq

            def drains(mt, pts):
                ms = slice(mt * P, (mt + 1) * P)
                for c in range(NCH):
                    yt = ypool.tile([P, CHW], bf16, name="yt", tag="yt")
                    nc.scalar.activation(
                        yt[:], pts[c][:], mybir.ActivationFunctionType.Copy
                    )
                    # y via SWDGE: keeps the HWDGE DMA lanes free of
                    # stores whose completion trails the matmul stream
                    # (shared-lane FIFO order would stall the x
                    # transposes behind them).
                    nc.gpsimd.dma_start(y_d[ms, c * CHW:(c + 1) * CHW], yt[:])

            # ---- software-pipelined main loop: x-prep runs one m-tile
            # ahead of its matmuls; PSUM drains trail one m-tile so their
            # matmul-completion waits never block the scalar engine's
            # x-load queue.
            xT_pipe = [x_prep(0)]
            if MT > 1:
                xT_pipe.append(x_prep(1))
            for kb in range(KB):
                w_prep(kb)
            prev = None       # (mt, pts) awaiting drain
            for mt in range(MT):
                xTq = xT_pipe.pop(0)
                if mt + 2 < MT:
                    xT_pipe.append(x_prep(mt + 2))
                if prev is not None:
                    drains(*prev)
                pts = [
                    psum.tile([P, CHW], mybir.dt.float32, name=f"pt{c}", tag="pt")
                    for c in range(NCH)
                ]
                for kb in range(KB):
                    q, kbl = divmod(kb, KH)
                    wv = wTs[kb][:].rearrange("p a b -> p (a b)")
                    for c in range(NCH):
                        nc.tensor.matmul(
                            pts[c][:],
                            xTq[q][:, kbl, :],
                            wv[:, c * CHW:(c + 1) * CHW],
                            start=(kb == 0),
                            stop=(kb == KB - 1),
                        )
                prev = (mt, pts)
            drains(*prev)

    nc.compile()
    return nc


def _core_inputs(x, weight, ws, c, nsh=NSH, nb=NB):
    """Shard + lay out inputs for core c. Layout/dtype transforms only:
    the fp8 cast of w/2 is exact (all values <= 224)."""
    import ml_dtypes

    wsl = weight[c * nsh:(c + 1) * nsh]
    wt8 = np.ascontiguousarray(
        (wsl.T * np.float32(0.5)).astype(ml_dtypes.float8_e4m3)
    )
    return {
        "x": x,
        "wt8": wt8,
        "ws": np.ascontiguousarray(ws[c * nb:(c + 1) * nb]),
    }


def kernel(x, weight, weight_scale_inv):
    from concourse.bass_utils import run_bass_kernel_spmd

    if "nc" not in _NC_CACHE:
        _NC_CACHE["nc"] = _build()
    nc = _NC_CACHE["nc"]

    x = np.ascontiguousarray(np.asarray(x, dtype=np.float32))
    weight = np.asarray(weight, dtype=np.float32)
    ws = np.asarray(weight_scale_inv, dtype=np.float32)

    in_maps = [_core_inputs(x, weight, ws, c) for c in range(NCORES)]
    res = run_bass_kernel_spmd(nc, in_maps, list(range(NCORES)))
    y = np.concatenate(
        [np.asarray(res.results[c]["y"]) for c in range(NCORES)], axis=1
    )
    return y.astype(np.float32)
